# revision 13
# baseline (speedup 1.0000x reference)
"""Trainium2 Bass kernel for nn_LowpassDetector.

Computes: power = re^2 + im^2, 5-tap FIR (b), order-4 IIR recurrence (a)
along time, for signal [2, T=16384, B=2048] -> y [T, B].

Strategy: the FIR+IIR cascade is LTI with all poles at radius <= 0.758,
so the combined impulse response decays below 1e-15 within 128 taps.
The whole filter is therefore exactly (to fp32) a block-Toeplitz matmul:
  y_blk[b] = T0 @ x_blk[b] + T1 @ x_blk[b-1]     (b >= 1)
  y_blk[0] = L0 @ x_blk[0]
with L0 the exact 128x128 operator of the reference recurrence
(including its "first 5 samples pass through" initial condition), built
on the host in float64. Channels (2048) are sharded 256 per core across
8 cores; time blocks of 128 map to the TensorEngine contraction dim.

v6 design (from v5's ~91-98 us; trace-driven):
- All I/O fp16 (fp8-e3m4/e4m3 input was simulated on host and fails the
  2e-2 max-rel budget at 3-6.6e-2: the metric is tail-dominated and
  fp8's coarse ulp at large |x| survives the filter). 25.3 MB/core at
  the measured ~420 GB/s sustained = ~60 us hard DMA floor.
- v5's trace: ~7 us fixed preamble, 16 serial input-DMA issues at
  ~0.6 us each, a 23 us tail at 1/3 rate where stores trailed per-SB
  compute, and ~8 us fixed teardown (full semaphore-file clear). v6:
  * The whole input (128 KB/partition) is SBUF-resident: 9 chunk DMAs
    (7x2-SB + 2x1-SB) all issued up-front on the Sync HWDGE queue, so
    input streams back-to-back from ~7.6 us with no further issue
    dependencies. Final chunks are small to shorten the drain tail.
  * Engine split by measured per-column cost (DVE 2x fp16 tensor_tensor
    0.52 ns/col, ACT 0.83, GPS ~2.1): DVE does re^2 (non-in-place, 2x)
    and the power add (2x) chunk-wide; ACT squares 2560/4096 im cols
    and drains 3/4 of each PSUM tile (it sits closest to PSUM); GPS
    squares the other 1536 im cols and issues the lag-2 chunk stores
    on its SWDGE ring. ~5.5-6 us of engine time per 2-SB chunk, under
    the ~6.5 us/chunk global engine budget, so stores are produced
    fast enough to keep the SDMA engines fed to the end.
  * Drains are issued one chunk late (after the next chunk's forward
    elementwise ops) so their PE waits are free; stores lag two chunks.
  * Each chunk's xh has a C-wide margin holding the previous chunk's
    last block (one 256-col copy per chunk) so every matmul rhs --
    including the cross-superbatch T1 operand -- is one contiguous AP.
- PSUM rules (kept from v5, learned the hard way): a matmul output
  region must not straddle a 2 KB bank boundary, and each half-bank
  holds exactly one accumulation group, opened once and closed once.
"""

import sys
from contextlib import ExitStack

import numpy as np

for _p in ("/opt/trn_rl_repo",):
    if _p not in sys.path:
        sys.path.insert(0, _p)

import concourse.bass as bass  # noqa: E402
import concourse.tile as tile  # noqa: E402
from concourse import bacc, mybir  # noqa: E402
from concourse.bass_utils import run_bass_kernel_spmd  # noqa: E402

T, B, NCORES = 16384, 2048, 8
BL = 128                # time-block size (= PE contraction dim)
NB = T // BL            # 128 time blocks
C = B // NCORES         # 256 channels per core
SBW = 8                 # time blocks per superbatch
NSB = NB // SBW         # 16 superbatches
CHUNKS = (2, 2, 2, 2, 2, 2, 2, 1, 1)   # superbatches per chunk
SBC = SBW * C           # 2048 columns per superbatch (one block-row)
F32 = mybir.dt.float32
F16 = mybir.dt.float16

TRACE = False           # set by test harness for NTFF profiling
LAST_RESULTS = None     # BassKernelResults of the last run (for profiling)

_program_cache = {}


def _reference_operator(bb, aa, n):
    """Exact linear operator of the reference filter on n samples (float64).

    Columns are responses to basis vectors; replicates the reference
    semantics: xf = zero-padded cross-correlation with b, first 5 outputs
    pass through, recurrence y[t] = xf[t] - sum_j a_j y[t-j] from t=5.
    """
    x = np.eye(n)
    xp = np.concatenate([np.zeros((4, n)), x], 0)
    xf = sum(bb[k] * xp[k:k + n] for k in range(5))
    y = xf.copy()
    at = aa[:4]
    for t in range(5, n):
        y[t] = xf[t] - (at[0] * y[t - 4] + at[1] * y[t - 3]
                        + at[2] * y[t - 2] + at[3] * y[t - 1])
    return y


def _build_mats(b32, a32):
    """Returns [BL, 3*BL] fp16: the three lhsT operands packed so the
    weights load with a single contiguous DMA (768 B per partition)."""
    bb = np.asarray(b32, np.float64)
    aa = np.asarray(a32, np.float64)
    M = _reference_operator(bb, aa, 3 * BL)
    L0 = M[0:BL, 0:BL]
    T0 = M[2 * BL:3 * BL, 2 * BL:3 * BL]
    T1 = M[2 * BL:3 * BL, BL:2 * BL]
    # truncation + init-transient leakage must be below fp32 noise
    leak = np.abs(M[2 * BL:3 * BL, 0:BL]).max()
    dev = max(np.abs(M[BL:2 * BL, BL:2 * BL] - T0).max(),
              np.abs(M[BL:2 * BL, 0:BL] - T1).max())
    assert leak < 1e-9 and dev < 1e-9, (leak, dev)

    w = np.empty((BL, 3 * BL), np.float16)
    for j, W in enumerate((L0, T0, T1)):
        w[:, j * BL:(j + 1) * BL] = W.T.astype(np.float16)  # lhsT = W.T
    return np.ascontiguousarray(w)


def _chunk_starts():
    starts, s0 = [], 0
    for L in CHUNKS:
        starts.append(s0)
        s0 += L
    assert s0 == NSB
    return starts


def _build_program():
    nc = bacc.Bacc("TRN2", target_bir_lowering=False, debug=False)
    # input cols per chunk: [re: L*SBC][im: L*SBC], chunk-major
    sig = nc.dram_tensor("sig", [BL, NSB * 2 * SBC], F16,
                         kind="ExternalInput").ap()
    wd = nc.dram_tensor("w", [BL, 3 * BL], F16, kind="ExternalInput").ap()
    yd = nc.dram_tensor("y", [BL, NSB * SBC], F16,
                        kind="ExternalOutput").ap()

    starts = _chunk_starts()
    base = [2 * SBC * s for s in starts]

    with tile.TileContext(nc) as tc, ExitStack() as ctx:
        wpool = ctx.enter_context(tc.tile_pool(name="w", bufs=1))
        wsb = wpool.tile([BL, 3 * BL], F16, tag="w", name="w_sb")
        nc.sync.dma_start(wsb[:], wd)
        w = {"l0": wsb[:, 0:BL], "t0": wsb[:, BL:2 * BL],
             "t1": wsb[:, 2 * BL:3 * BL]}

        sigpool = ctx.enter_context(tc.tile_pool(name="sig", bufs=1))
        re2pool = ctx.enter_context(tc.tile_pool(name="re2", bufs=1))
        im2pool = ctx.enter_context(tc.tile_pool(name="im2", bufs=2))
        xhpool = ctx.enter_context(tc.tile_pool(name="xh", bufs=2))
        yspool = ctx.enter_context(tc.tile_pool(name="ys", bufs=3))
        pspool = ctx.enter_context(tc.tile_pool(name="ps", bufs=2,
                                                space="PSUM"))

        sig_sb = sigpool.tile([BL, NSB * 2 * SBC], F16, tag="sig",
                              name="sig_sb")
        # all input DMAs issued up-front: back-to-back on the Sync ring
        for c, L in enumerate(CHUNKS):
            sp = slice(base[c], base[c] + 2 * L * SBC)
            nc.sync.dma_start(sig_sb[:, sp], sig[:, sp])

        def mm(ps_ap, wt, rhs_ap, start=False, stop=False):
            nc.tensor.matmul(ps_ap, w[wt], rhs_ap, start=start, stop=stop)

        MAXW = 2 * SBC                      # widest chunk (L=2) in cols

        def drain(ent):
            """PSUM -> ys fp16, all on ACT (it owns the PSUM port here)."""
            s, l, ps0, ps1, ys = ent
            o = l * SBC
            nc.scalar.activation(ys[:, o:o + 4 * C], ps0[:],
                                 mybir.ActivationFunctionType.Copy)
            nc.scalar.activation(ys[:, o + 4 * C:o + 8 * C], ps1[:],
                                 mybir.ActivationFunctionType.Copy)

        def im_slices(c, L):
            """Per-SB im^2 column split: DVE [0:2C], ACT [2C:4C],
            GPS [4C:8C] of each superbatch's 8C im columns."""
            out = []
            for l in range(L):
                o = base[c] + L * SBC + l * SBC
                out.append((o, l * SBC))
            return out

        def gps_prefetch(c, L, im2):
            # GPS squares its [4C:8C] share of each SB one chunk early
            for o, d in im_slices(c, L):
                nc.gpsimd.tensor_mul(im2[:, d + 4 * C:d + 8 * C],
                                     sig_sb[:, o + 4 * C:o + 8 * C],
                                     sig_sb[:, o + 4 * C:o + 8 * C])

        prev_xh = None                      # (tile, L) of previous chunk
        prev_pend = []                      # drain entries of chunk c-1
        store_q = []                        # [(ys_ap, dram col slice)]
        im2_cur = im2pool.tile([BL, MAXW], F16, tag="im2", name="im2_0")
        gps_prefetch(0, CHUNKS[0], im2_cur)        # bootstrap chunk 0
        for c, L in enumerate(CHUNKS):
            W2 = L * SBC
            im2 = im2_cur

            re2 = re2pool.tile([BL, MAXW], F16, tag="re2")
            xh = xhpool.tile([BL, C + MAXW], F16, tag="xh")
            ys = yspool.tile([BL, MAXW], F16, tag="ys")

            # chunk margin: previous chunk's last block -> xh[:, 0:C]
            # (GPS, ahead of its prefetch so PE's T1 is never blocked)
            if c > 0:
                pxh, pl = prev_xh
                nc.gpsimd.tensor_copy(xh[:, 0:C],
                                      pxh[:, pl * SBC:pl * SBC + C])
            # ACT squares its im^2 share of THIS chunk first thing
            for o, d in im_slices(c, L):
                nc.scalar.activation(im2[:, d + 2 * C:d + 4 * C],
                                     sig_sb[:, o + 2 * C:o + 4 * C],
                                     mybir.ActivationFunctionType.Square)
            # GPS prefetches its share of the NEXT chunk
            if c + 1 < len(CHUNKS):
                im2_next = im2pool.tile([BL, MAXW], F16, tag="im2",
                                        name="im2_%d" % (c + 1))
                gps_prefetch(c + 1, CHUNKS[c + 1], im2_next)
            else:
                im2_next = None

            # per-SB: DVE re^2 (2x), its im^2 sliver, the power add
            # (2x), then the matmuls -- PE starts mid-chunk
            cur_pend = []
            for l in range(L):
                s = starts[c] + l
                ore = base[c] + l * SBC
                oim = base[c] + L * SBC + l * SBC
                d = l * SBC
                nc.vector.tensor_mul(re2[:, d:d + SBC],
                                     sig_sb[:, ore:ore + SBC],
                                     sig_sb[:, ore:ore + SBC])
                nc.vector.tensor_mul(im2[:, d:d + 2 * C],
                                     sig_sb[:, oim:oim + 2 * C],
                                     sig_sb[:, oim:oim + 2 * C])
                nc.vector.tensor_add(xh[:, C + d:C + d + SBC],
                                     re2[:, d:d + SBC],
                                     im2[:, d:d + SBC])
                ps0 = pspool.tile([BL, 4 * C], F32, tag="ps0",
                                  name="ps0_%d" % s)
                ps1 = pspool.tile([BL, 4 * C], F32, tag="ps1",
                                  name="ps1_%d" % s)
                # 9C view: [0:C] = previous block (margin or in-chunk),
                # [C:9C] = this superbatch's 8 blocks
                xb = xh[:, l * SBC:l * SBC + SBC + C]

                if s == 0:
                    # block 0: exact-init operator L0, no cross term;
                    # every region is a half-bank with exactly one
                    # accumulation group, opened once and closed once.
                    mm(ps0[:, 0:C], "l0", xb[:, C:2 * C],
                       start=True, stop=True)
                    mm(ps0[:, C:2 * C], "t0", xb[:, 2 * C:3 * C],
                       start=True)
                    mm(ps0[:, 2 * C:4 * C], "t0", xb[:, 3 * C:5 * C],
                       start=True)
                    mm(ps1[:, 0:2 * C], "t0", xb[:, 5 * C:7 * C],
                       start=True)
                    mm(ps1[:, 2 * C:4 * C], "t0", xb[:, 7 * C:9 * C],
                       start=True)
                    mm(ps0[:, C:2 * C], "t1", xb[:, C:2 * C], stop=True)
                    mm(ps0[:, 2 * C:4 * C], "t1", xb[:, 2 * C:4 * C],
                       stop=True)
                    mm(ps1[:, 0:2 * C], "t1", xb[:, 4 * C:6 * C],
                       stop=True)
                    mm(ps1[:, 2 * C:4 * C], "t1", xb[:, 6 * C:8 * C],
                       stop=True)
                else:
                    for q, ps in enumerate((ps0, ps1)):
                        o = 4 * q * C
                        mm(ps[:, 0:2 * C], "t0", xb[:, C + o:3 * C + o],
                           start=True)
                        mm(ps[:, 2 * C:4 * C], "t0",
                           xb[:, 3 * C + o:5 * C + o], start=True)
                    for q, ps in enumerate((ps0, ps1)):
                        o = 4 * q * C
                        mm(ps[:, 0:2 * C], "t1", xb[:, o:2 * C + o],
                           stop=True)
                        mm(ps[:, 2 * C:4 * C], "t1",
                           xb[:, 2 * C + o:4 * C + o], stop=True)
                cur_pend.append((s, l, ps0, ps1, ys))

            # drains for the previous chunk, issued at the tail of this
            # chunk's DVE/ACT streams: their matmuls finished while this
            # chunk's squares/add ran, so the waits cost nothing
            for ent in prev_pend:
                drain(ent)
            # lag-1 store on the idle Sync ring, issued after its drains
            # (program order = dependency order for tile)
            if store_q:
                ys_ap, cols = store_q.pop(0)
                nc.sync.dma_start(yd[:, cols], ys_ap)
            prev_pend = cur_pend
            prev_xh = (xh, L)
            im2_cur = im2_next
            store_q.append((ys[:, 0:W2],
                            slice(starts[c] * SBC, (starts[c] + L) * SBC)))

        # tail: drain the last chunk, then flush its store, split across
        # both DMA paths for latency
        for ent in prev_pend:
            drain(ent)
        ys_ap, cols = store_q.pop(0)
        n = cols.stop - cols.start
        nc.sync.dma_start(yd[:, cols.start:cols.start + n // 2],
                          ys_ap[:, 0:n // 2])
        nc.gpsimd.dma_start(yd[:, cols.start + n // 2:cols.stop],
                            ys_ap[:, n // 2:n])

    nc.compile()
    return nc


def kernel(signal, b, a):
    global LAST_RESULTS
    signal = np.asarray(signal)
    assert signal.shape == (2, T, B), signal.shape

    wmat = _build_mats(np.asarray(b), np.asarray(a))

    if "prog" not in _program_cache:
        _program_cache["prog"] = _build_program()
    nc = _program_cache["prog"]

    starts = _chunk_starts()
    # pack to per-core chunk-major fp16 layout:
    # [core, p, chunk{ re[l,b,ch] | im[l,b,ch] }]
    x = signal.reshape(2, NSB, SBW, BL, NCORES, C)
    parts = []
    for c, L in enumerate(CHUNKS):
        xs = x[:, starts[c]:starts[c] + L]        # [2, L, SBW, BL, 8, C]
        parts.append(xs.transpose(4, 3, 0, 1, 2, 5).reshape(
            NCORES, BL, 2 * L * SBW * C))
    pk = np.ascontiguousarray(np.concatenate(parts, axis=2),
                              dtype=np.float16)   # [8, BL, NSB*2*SBC]

    in_maps = [{"sig": pk[c], "w": wmat} for c in range(NCORES)]

    res = run_bass_kernel_spmd(nc, in_maps, core_ids=list(range(NCORES)),
                               trace=TRACE)
    LAST_RESULTS = res

    out = np.empty((T, B), np.float32)
    for c in range(NCORES):
        yc = np.asarray(res.results[c]["y"])      # [BL, NSB*SBC]
        yc = yc.reshape(BL, NSB, SBW, C).transpose(1, 2, 0, 3)
        out[:, c * C:(c + 1) * C] = yc.reshape(T, C).astype(np.float32)
    return out


# revision 15
# speedup vs baseline: 1.1243x; 1.1243x over previous
"""Trainium2 Bass kernel for nn_LowpassDetector.

Computes: power = re^2 + im^2, 5-tap FIR (b), order-4 IIR recurrence (a)
along time, for signal [2, T=16384, B=2048] -> y [T, B].

Strategy: the FIR+IIR cascade is LTI with all poles at radius <= 0.758,
so the combined impulse response decays below 1e-15 within 128 taps.
The whole filter is therefore exactly (to fp32) a block-Toeplitz matmul:
  y_blk[b] = T0 @ x_blk[b] + T1 @ x_blk[b-1]     (b >= 1)
  y_blk[0] = L0 @ x_blk[0]
with L0 the exact 128x128 operator of the reference recurrence
(including its "first 5 samples pass through" initial condition), built
on the host in float64. Channels (2048) are sharded 256 per core across
8 cores; time blocks of 128 map to the TensorEngine contraction dim.

v6 design (from v5's ~91-98 us; trace-driven):
- All I/O fp16 (fp8-e3m4/e4m3 input was simulated on host and fails the
  2e-2 max-rel budget at 3-6.6e-2: the metric is tail-dominated and
  fp8's coarse ulp at large |x| survives the filter). 25.3 MB/core at
  the measured ~420 GB/s sustained = ~60 us hard DMA floor.
- v5's trace: ~7 us fixed preamble, 16 serial input-DMA issues at
  ~0.6 us each, a 23 us tail at 1/3 rate where stores trailed per-SB
  compute, and ~8 us fixed teardown (full semaphore-file clear). v6:
  * The whole input (128 KB/partition) is SBUF-resident: 9 chunk DMAs
    (7x2-SB + 2x1-SB) all issued up-front on the Sync HWDGE queue, so
    input streams back-to-back from ~7.6 us with no further issue
    dependencies. Final chunks are small to shorten the drain tail.
  * Engine split by measured per-column cost (DVE 2x fp16 tensor_tensor
    0.52 ns/col, ACT 0.83, GPS ~2.1): DVE does re^2 (non-in-place, 2x)
    and the power add (2x) chunk-wide; ACT squares 2560/4096 im cols
    and drains 3/4 of each PSUM tile (it sits closest to PSUM); GPS
    squares the other 1536 im cols and issues the lag-2 chunk stores
    on its SWDGE ring. ~5.5-6 us of engine time per 2-SB chunk, under
    the ~6.5 us/chunk global engine budget, so stores are produced
    fast enough to keep the SDMA engines fed to the end.
  * Drains are issued one chunk late (after the next chunk's forward
    elementwise ops) so their PE waits are free; stores lag two chunks.
  * Each chunk's xh has a C-wide margin holding the previous chunk's
    last block (one 256-col copy per chunk) so every matmul rhs --
    including the cross-superbatch T1 operand -- is one contiguous AP.
- PSUM rules (kept from v5, learned the hard way): a matmul output
  region must not straddle a 2 KB bank boundary, and each half-bank
  holds exactly one accumulation group, opened once and closed once.
"""

import sys
from contextlib import ExitStack

import numpy as np

for _p in ("/opt/trn_rl_repo",):
    if _p not in sys.path:
        sys.path.insert(0, _p)

import concourse.bass as bass  # noqa: E402
import concourse.tile as tile  # noqa: E402
from concourse import bacc, mybir  # noqa: E402
from concourse.bass_utils import run_bass_kernel_spmd  # noqa: E402

T, B, NCORES = 16384, 2048, 8
BL = 128                # time-block size (= PE contraction dim)
NB = T // BL            # 128 time blocks
C = B // NCORES         # 256 channels per core
SBW = 8                 # time blocks per superbatch
NSB = NB // SBW         # 16 superbatches
CHUNKS = (2, 2, 2, 2, 2, 2, 2, 1, 1)   # superbatches per chunk
SBC = SBW * C           # 2048 columns per superbatch (one block-row)
F32 = mybir.dt.float32
F16 = mybir.dt.float16

TRACE = False           # set by test harness for NTFF profiling
LAST_RESULTS = None     # BassKernelResults of the last run (for profiling)

_program_cache = {}


def _reference_operator(bb, aa, n):
    """Exact linear operator of the reference filter on n samples (float64).

    Columns are responses to basis vectors; replicates the reference
    semantics: xf = zero-padded cross-correlation with b, first 5 outputs
    pass through, recurrence y[t] = xf[t] - sum_j a_j y[t-j] from t=5.
    """
    x = np.eye(n)
    xp = np.concatenate([np.zeros((4, n)), x], 0)
    xf = sum(bb[k] * xp[k:k + n] for k in range(5))
    y = xf.copy()
    at = aa[:4]
    for t in range(5, n):
        y[t] = xf[t] - (at[0] * y[t - 4] + at[1] * y[t - 3]
                        + at[2] * y[t - 2] + at[3] * y[t - 1])
    return y


def _build_mats(b32, a32):
    """Returns [BL, 3*BL] fp16: the three lhsT operands packed so the
    weights load with a single contiguous DMA (768 B per partition)."""
    bb = np.asarray(b32, np.float64)
    aa = np.asarray(a32, np.float64)
    M = _reference_operator(bb, aa, 3 * BL)
    L0 = M[0:BL, 0:BL]
    T0 = M[2 * BL:3 * BL, 2 * BL:3 * BL]
    T1 = M[2 * BL:3 * BL, BL:2 * BL]
    # truncation + init-transient leakage must be below fp32 noise
    leak = np.abs(M[2 * BL:3 * BL, 0:BL]).max()
    dev = max(np.abs(M[BL:2 * BL, BL:2 * BL] - T0).max(),
              np.abs(M[BL:2 * BL, 0:BL] - T1).max())
    assert leak < 1e-9 and dev < 1e-9, (leak, dev)

    w = np.empty((BL, 3 * BL), np.float16)
    for j, W in enumerate((L0, T0, T1)):
        w[:, j * BL:(j + 1) * BL] = W.T.astype(np.float16)  # lhsT = W.T
    return np.ascontiguousarray(w)


def _chunk_starts():
    starts, s0 = [], 0
    for L in CHUNKS:
        starts.append(s0)
        s0 += L
    assert s0 == NSB
    return starts


def _build_program():
    nc = bacc.Bacc("TRN2", target_bir_lowering=False, debug=False)
    # input cols per chunk: [re: L*SBC][im: L*SBC], chunk-major
    sig = nc.dram_tensor("sig", [BL, NSB * 2 * SBC], F16,
                         kind="ExternalInput").ap()
    wd = nc.dram_tensor("w", [BL, 3 * BL], F16, kind="ExternalInput").ap()
    yd = nc.dram_tensor("y", [BL, NSB * SBC], F16,
                        kind="ExternalOutput").ap()

    starts = _chunk_starts()
    base = [2 * SBC * s for s in starts]
    NCH = len(CHUNKS)

    with tile.TileContext(nc) as tc, ExitStack() as ctx:
        wpool = ctx.enter_context(tc.tile_pool(name="w", bufs=1))
        wsb = wpool.tile([BL, 3 * BL], F16, tag="w", name="w_sb")
        nc.sync.dma_start(wsb[:], wd)
        w = {"l0": wsb[:, 0:BL], "t0": wsb[:, BL:2 * BL],
             "t1": wsb[:, 2 * BL:3 * BL]}

        sigpool = ctx.enter_context(tc.tile_pool(name="sig", bufs=1))
        re2pool = ctx.enter_context(tc.tile_pool(name="re2", bufs=1))
        im2pool = ctx.enter_context(tc.tile_pool(name="im2", bufs=2))
        xhpool = ctx.enter_context(tc.tile_pool(name="xh", bufs=2))
        yspool = ctx.enter_context(tc.tile_pool(name="ys", bufs=3))
        pspool = ctx.enter_context(tc.tile_pool(name="ps", bufs=2,
                                                space="PSUM"))

        sig_sb = sigpool.tile([BL, NSB * 2 * SBC], F16, tag="sig",
                              name="sig_sb")
        # all input DMAs issued up-front: back-to-back on the Sync ring
        for c, L in enumerate(CHUNKS):
            sp = slice(base[c], base[c] + 2 * L * SBC)
            nc.sync.dma_start(sig_sb[:, sp], sig[:, sp])

        def mm(ps_ap, wt, rhs_ap, start=False, stop=False):
            nc.tensor.matmul(ps_ap, w[wt], rhs_ap, start=start, stop=stop)

        MAXW = 2 * SBC                      # widest chunk (L=2) in cols

        def gps_prefetch(c, im2):
            # GPS squares the back half of chunk c's im columns early
            L = CHUNKS[c]
            o = base[c] + L * SBC
            W2 = L * SBC
            nc.gpsimd.tensor_mul(im2[:, W2 // 2:W2],
                                 sig_sb[:, o + W2 // 2:o + W2],
                                 sig_sb[:, o + W2 // 2:o + W2])

        def drain(ent):
            """PSUM -> ys fp16, all on ACT (it owns the PSUM port)."""
            s, l, ps0, ps1, ys = ent
            o = l * SBC
            nc.scalar.activation(ys[:, o:o + 4 * C], ps0[:],
                                 mybir.ActivationFunctionType.Copy)
            nc.scalar.activation(ys[:, o + 4 * C:o + 8 * C], ps1[:],
                                 mybir.ActivationFunctionType.Copy)

        prev_xh = None                      # (tile, L) of previous chunk
        pend = []                           # drain entries, FIFO by chunk
        store_q = []                        # [(ys_ap, dram col slice)]
        im2_cur = im2pool.tile([BL, MAXW], F16, tag="im2", name="im2_0")
        gps_prefetch(0, im2_cur)                   # bootstrap chunk 0
        for c, L in enumerate(CHUNKS):
            W2 = L * SBC
            im2 = im2_cur
            oim = base[c] + W2

            re2 = re2pool.tile([BL, MAXW], F16, tag="re2")
            xh = xhpool.tile([BL, MAXW], F16, tag="xh")
            ys = yspool.tile([BL, MAXW], F16, tag="ys")

            # ACT squares its middle im^2 share of THIS chunk up front;
            # DVE its front sliver; GPS's back half was prefetched
            nc.scalar.activation(im2[:, W2 // 8:W2 // 2],
                                 sig_sb[:, oim + W2 // 8:oim + W2 // 2],
                                 mybir.ActivationFunctionType.Square)
            if c + 1 < NCH:
                im2_next = im2pool.tile([BL, MAXW], F16, tag="im2",
                                        name="im2_%d" % (c + 1))
                gps_prefetch(c + 1, im2_next)
            else:
                im2_next = None

            # DVE chunk-wide: re^2 (2x), im^2 sliver, power add (2x)
            nc.vector.tensor_mul(re2[:, 0:W2],
                                 sig_sb[:, base[c]:base[c] + W2],
                                 sig_sb[:, base[c]:base[c] + W2])
            nc.vector.tensor_mul(im2[:, 0:W2 // 8],
                                 sig_sb[:, oim:oim + W2 // 8],
                                 sig_sb[:, oim:oim + W2 // 8])
            nc.vector.tensor_add(xh[:, 0:W2], re2[:, 0:W2],
                                 im2[:, 0:W2])

            # matmuls; blocks of superbatch l live at xh[l*SBC + i*C]
            for l in range(L):
                s = starts[c] + l
                bs = l * SBC

                def b(i, n=1):
                    return xh[:, bs + i * C:bs + (i + n) * C]

                ps0 = pspool.tile([BL, 4 * C], F32, tag="ps0",
                                  name="ps0_%d" % s)
                ps1 = pspool.tile([BL, 4 * C], F32, tag="ps1",
                                  name="ps1_%d" % s)
                if s == 0:
                    # exact-init operator L0 for block 0, no cross term
                    mm(ps0[:, 0:C], "l0", b(0), start=True, stop=True)
                    mm(ps0[:, C:2 * C], "t0", b(1), start=True)
                    mm(ps0[:, 2 * C:4 * C], "t0", b(2, 2), start=True)
                    mm(ps1[:, 0:2 * C], "t0", b(4, 2), start=True)
                    mm(ps1[:, 2 * C:4 * C], "t0", b(6, 2), start=True)
                    mm(ps0[:, C:2 * C], "t1", b(0), stop=True)
                    mm(ps0[:, 2 * C:4 * C], "t1", b(1, 2), stop=True)
                    mm(ps1[:, 0:2 * C], "t1", b(3, 2), stop=True)
                    mm(ps1[:, 2 * C:4 * C], "t1", b(5, 2), stop=True)
                elif l == 0:
                    # cross-chunk T1 operand comes straight from the
                    # previous chunk's xh tile, so ps0's first bank is
                    # split into two half-bank groups (like the s==0
                    # pattern) to keep every rhs a contiguous AP
                    # bank 0 holds two half-bank groups: close [0:C]
                    # BEFORE opening [C:2C] (start=True clears the
                    # whole bank's has_written bits)
                    pxh, pl = prev_xh
                    prevC = pxh[:, pl * SBC - C:pl * SBC]
                    mm(ps0[:, 0:C], "t0", b(0), start=True)
                    mm(ps0[:, 0:C], "t1", prevC, stop=True)
                    mm(ps0[:, C:2 * C], "t0", b(1), start=True)
                    mm(ps0[:, 2 * C:4 * C], "t0", b(2, 2), start=True)
                    mm(ps1[:, 0:2 * C], "t0", b(4, 2), start=True)
                    mm(ps1[:, 2 * C:4 * C], "t0", b(6, 2), start=True)
                    mm(ps0[:, C:2 * C], "t1", b(0), stop=True)
                    mm(ps0[:, 2 * C:4 * C], "t1", b(1, 2), stop=True)
                    mm(ps1[:, 0:2 * C], "t1", b(3, 2), stop=True)
                    mm(ps1[:, 2 * C:4 * C], "t1", b(5, 2), stop=True)
                else:
                    for q, ps in enumerate((ps0, ps1)):
                        o = 4 * q
                        mm(ps[:, 0:2 * C], "t0", b(o, 2), start=True)
                        mm(ps[:, 2 * C:4 * C], "t0", b(o + 2, 2),
                           start=True)
                    for q, ps in enumerate((ps0, ps1)):
                        o = 4 * q
                        mm(ps[:, 0:2 * C], "t1", b(o - 1, 2), stop=True)
                        mm(ps[:, 2 * C:4 * C], "t1", b(o + 1, 2),
                           stop=True)
                pend.append((s, l, ps0, ps1, ys))

            # drains + store for chunk c-2 (PE finished it last slot)
            if c >= 2:
                Lp = CHUNKS[c - 2]
                ents, pend = pend[:Lp], pend[Lp:]
                for ent in ents:
                    drain(ent)
                ys_ap, cols = store_q.pop(0)
                nc.sync.dma_start(yd[:, cols], ys_ap)

            prev_xh = (xh, L)
            im2_cur = im2_next
            store_q.append((ys[:, 0:W2],
                            slice(starts[c] * SBC, (starts[c] + L) * SBC)))

        # tail: drain + store the last two chunks
        for k, c in enumerate((NCH - 2, NCH - 1)):
            Lp = CHUNKS[c]
            ents, pend = pend[:Lp], pend[Lp:]
            for ent in ents:
                drain(ent)
            ys_ap, cols = store_q.pop(0)
            if k == 0:
                nc.sync.dma_start(yd[:, cols], ys_ap)
            else:
                # final store split across both DMA paths for latency
                n = cols.stop - cols.start
                nc.sync.dma_start(yd[:, cols.start:cols.start + n // 2],
                                  ys_ap[:, 0:n // 2])
                nc.gpsimd.dma_start(yd[:, cols.start + n // 2:cols.stop],
                                    ys_ap[:, n // 2:n])
        assert not pend and not store_q

    nc.compile()
    return nc


def kernel(signal, b, a):
    global LAST_RESULTS
    signal = np.asarray(signal)
    assert signal.shape == (2, T, B), signal.shape

    wmat = _build_mats(np.asarray(b), np.asarray(a))

    if "prog" not in _program_cache:
        _program_cache["prog"] = _build_program()
    nc = _program_cache["prog"]

    starts = _chunk_starts()
    # pack to per-core chunk-major fp16 layout:
    # [core, p, chunk{ re[l,b,ch] | im[l,b,ch] }]
    x = signal.reshape(2, NSB, SBW, BL, NCORES, C)
    parts = []
    for c, L in enumerate(CHUNKS):
        xs = x[:, starts[c]:starts[c] + L]        # [2, L, SBW, BL, 8, C]
        parts.append(xs.transpose(4, 3, 0, 1, 2, 5).reshape(
            NCORES, BL, 2 * L * SBW * C))
    pk = np.ascontiguousarray(np.concatenate(parts, axis=2),
                              dtype=np.float16)   # [8, BL, NSB*2*SBC]

    in_maps = [{"sig": pk[c], "w": wmat} for c in range(NCORES)]

    res = run_bass_kernel_spmd(nc, in_maps, core_ids=list(range(NCORES)),
                               trace=TRACE)
    LAST_RESULTS = res

    out = np.empty((T, B), np.float32)
    for c in range(NCORES):
        yc = np.asarray(res.results[c]["y"])      # [BL, NSB*SBC]
        yc = yc.reshape(BL, NSB, SBW, C).transpose(1, 2, 0, 3)
        out[:, c * C:(c + 1) * C] = yc.reshape(T, C).astype(np.float32)
    return out
